# revision 1
# baseline (speedup 1.0000x reference)
"""Trainium2 Bass kernel for nn_MAD_72679436582977 (retrieval_knn).

For each edge endpoint (src/tgt of 1024 edges) and each of 4 heads: find the
8 nearest neighbors (excluding self) among 50000 nodes in a 32-d embedding
space, logits (q - e_k).f_q, dist |q - e_k|, softmax(1 - dist) over
16 neighbors + 8 sentinels, sigmoid of head-mean weighted sum.

Strategy: data-parallel over edges across 8 cores (128 edges/core, SPMD, no
collectives). Per core:
  - approximate distance GEMM s~[q, n] = 2 q.e_n - |e_n|^2 in float32r
    (full PE rate; ~1e-4 relative error), K=33 (32 dims + norm row),
    supers of 2048 nodes in PSUM;
  - per-super top-8 via DVE max8 + max_index directly from PSUM
    -> 200 approximate candidates/row;
  - approximate top-16 of the candidates (covers the exact top-9 with large
    margin: rank-9 to rank-16 value gaps >> f32r error);
  - indirect-DMA gather of the 16 candidate rows [embed(32) | norm | pad]
    and EXACT fp32 recompute of s_k = 2 q.e_k - |e_k|^2 on DVE
    (q itself is the gathered rank-1 row = self);
  - exact top-9 among the 16, drop rank-1 (self) -> exact neighbor set;
  - epilogue on-device: dist = sqrt(qn - s), weights exp(1-dist), logits via
    gathered embeds . field, softmax-ratio with sentinel mass, head mean,
    sigmoid.

Host only shards edges, lays out inputs, and concatenates the 8 per-core
outputs.
"""
import os
import sys

sys.path.insert(0, "/opt/trn_rl_repo")

import numpy as np

import concourse.bass as bass
import concourse.bacc as bacc
import concourse.mybir as mybir
from concourse import tile
from concourse.bass import IndirectOffsetOnAxis

F32 = mybir.dt.float32
F32R = mybir.dt.float32r
U32 = mybir.dt.uint32

N_HEADS = 4
N_NODES = 50000
DIM = 32
N_BATCH = 1024
N_SENT = 8
N_CORES = 8

EDGES_PER_CORE = N_BATCH // N_CORES          # 128
SUPER = 2048                                  # PSUM super-chunk (4 banks)
N_SUPERS = 25
LAST_W = 1024                                 # last super is half-width
N_PAD = SUPER * (N_SUPERS - 1) + LAST_W       # 50176
M_TILES = N_HEADS * 2                         # (head, src/tgt) tiles of 128 rows
KC = DIM + 1                                  # contraction: 32 dims + norm row
N_CAND = N_SUPERS * 8                         # 200 approx candidates per row
EW = DIM + 2                                  # gathered row: embed|norm|pad

LAST = {}


def _build_program():
    nc = bacc.Bacc(None, num_swdge_queues=2)

    rhs_d = nc.dram_tensor("rhs_aug", [N_HEADS, KC, N_PAD], F32R,
                           kind="ExternalInput")
    embn_d = nc.dram_tensor("embn", [N_HEADS * N_NODES, EW], F32,
                            kind="ExternalInput")
    qpack_d = nc.dram_tensor("qpack", [M_TILES, KC, 128], F32R,
                             kind="ExternalInput")
    aux_d = nc.dram_tensor("aux", [M_TILES, 128, DIM + 2], F32,
                           kind="ExternalInput")

    preds_d = nc.dram_tensor("preds", [128, 1], F32, kind="ExternalOutput")
    dbg_gid_d = nc.dram_tensor("dbg_gid", [M_TILES, 128, 16], U32,
                               kind="ExternalOutput")
    dbg_s_d = nc.dram_tensor("dbg_s", [M_TILES, 128, 8], F32,
                             kind="ExternalOutput")

    with tile.TileContext(nc) as tc:
        with tc.tile_pool(name="const", bufs=1) as cpool, \
             tc.tile_pool(name="qp", bufs=3) as qpool, \
             tc.tile_pool(name="rhs", bufs=4) as rpool, \
             tc.tile_pool(name="cand", bufs=3) as candp, \
             tc.tile_pool(name="small", bufs=3) as sp, \
             tc.tile_pool(name="acc", bufs=1) as accp, \
             tc.tile_pool(name="ps", bufs=2, space="PSUM") as psp:

            # constants
            iota_cand = cpool.tile([128, N_CAND], U32, tag="iota_cand")
            nc.gpsimd.iota(iota_cand[:], pattern=[[1, N_CAND]], base=0,
                           channel_multiplier=0)
            iota16 = cpool.tile([128, 16], U32, tag="iota16")
            nc.gpsimd.iota(iota16[:], pattern=[[1, 16]], base=0,
                           channel_multiplier=0)
            neg_inf8 = cpool.tile([128, 8], F32, tag="neg_inf8")
            nc.vector.memset(neg_inf8[:], -1e30)

            # per-head gid offsets: gid = h*N_NODES + j*SUPER + within
            ioff_h = []
            for h in range(N_HEADS):
                t = cpool.tile([128, N_SUPERS, 8], F32, tag=f"ioff{h}")
                nc.gpsimd.iota(t[:], pattern=[[SUPER, N_SUPERS], [0, 8]],
                               base=h * N_NODES, channel_multiplier=0,
                               allow_small_or_imprecise_dtypes=True)
                ioff_h.append(t)

            numneg_all = accp.tile([128, M_TILES], F32, tag="numneg")
            wsum_all = accp.tile([128, M_TILES], F32, tag="wsum")

            for m in range(M_TILES):
                h = m // 2
                q_s = qpool.tile([KC, 128], F32R, tag="q")
                nc.sync.dma_start(out=q_s[:], in_=qpack_d[m])
                aux_s = sp.tile([128, DIM + 2], F32, tag="aux")
                nc.sync.dma_start(out=aux_s[:], in_=aux_d[m])
                qn_s = aux_s[:, 0:1]
                qf_s = aux_s[:, 1:2]
                f_s = aux_s[:, 2:DIM + 2]

                cand_v = candp.tile([128, N_CAND], F32, tag="cv")
                cand_i = candp.tile([128, N_CAND], U32, tag="ci")

                for j in range(N_SUPERS):
                    w = SUPER if j < N_SUPERS - 1 else LAST_W
                    rhs_s = rpool.tile([KC, w], F32R, tag="rhs")
                    nc.sync.dma_start(
                        out=rhs_s[:], in_=rhs_d[h, :, j * SUPER:j * SUPER + w])
                    psum = psp.tile([128, w], F32, tag="ps")
                    for b in range(w // 512):
                        sl = slice(b * 512, (b + 1) * 512)
                        nc.tensor.matmul(psum[:, sl], q_s[:], rhs_s[:, sl],
                                         start=True, stop=True)
                    csl = slice(j * 8, (j + 1) * 8)
                    nc.vector.max(cand_v[:, csl], psum[:])
                    nc.vector.max_index(cand_i[:, csl], cand_v[:, csl], psum[:])

                # global gids (fp32; < 2^24 so exact)
                cand_g = candp.tile([128, N_CAND], F32, tag="cg")
                nc.vector.tensor_tensor(
                    out=cand_g[:],
                    in0=cand_i[:].rearrange("p (a b) -> p a b", b=8),
                    in1=ioff_h[h][:],
                    op=mybir.AluOpType.add)

                # approx top-16 (rank 1 = self by construction), first-8
                # extraction/gathers fire before the second max8 round so the
                # gather chain starts as early as possible
                m16 = sp.tile([128, 16], F32, tag="m16")
                wpos16 = sp.tile([128, 16], U32, tag="wpos16")
                wgid_f = sp.tile([128, 16], F32, tag="wgidf")
                wgid = sp.tile([128, 16], U32, tag="wgid")
                scratch = candp.tile([128, N_CAND], F32, tag="scr")
                gath = sp.tile([128, 16, EW], F32, tag="gath")

                nc.vector.max(m16[:, 0:8], cand_v[:])
                nc.vector.max_index(wpos16[:, 0:8], m16[:, 0:8], cand_v[:])
                for k in range(8):
                    nc.vector.scalar_tensor_tensor(
                        out=scratch[:], in0=iota_cand[:],
                        scalar=wpos16[:, k:k + 1], in1=cand_g[:],
                        op0=mybir.AluOpType.is_equal,
                        op1=mybir.AluOpType.mult,
                        accum_out=wgid_f[:, k:k + 1])
                    nc.vector.tensor_copy(wgid[:, k:k + 1], wgid_f[:, k:k + 1])
                    nc.gpsimd.indirect_dma_start(
                        out=gath[:, k], out_offset=None,
                        in_=embn_d[:],
                        in_offset=IndirectOffsetOnAxis(ap=wgid[:, k:k + 1],
                                                       axis=0))
                vrep = candp.tile([128, N_CAND], F32, tag="vrep")
                nc.vector.match_replace(vrep[:], m16[:, 0:8], cand_v[:], -1e30)
                nc.vector.max(m16[:, 8:16], vrep[:])
                nc.vector.max_index(wpos16[:, 8:16], m16[:, 8:16], vrep[:])
                for k in range(8, 16):
                    nc.vector.scalar_tensor_tensor(
                        out=scratch[:], in0=iota_cand[:],
                        scalar=wpos16[:, k:k + 1], in1=cand_g[:],
                        op0=mybir.AluOpType.is_equal,
                        op1=mybir.AluOpType.mult,
                        accum_out=wgid_f[:, k:k + 1])
                    nc.vector.tensor_copy(wgid[:, k:k + 1], wgid_f[:, k:k + 1])
                    nc.gpsimd.indirect_dma_start(
                        out=gath[:, k], out_offset=None,
                        in_=embn_d[:],
                        in_offset=IndirectOffsetOnAxis(ap=wgid[:, k:k + 1],
                                                       axis=0))

                # exact s_k = 2 q.e_k - |e_k|^2 ; q = gathered rank-1 row
                prod16 = sp.tile([128, 16, DIM], F32, tag="prod16")
                nc.vector.tensor_tensor(
                    out=prod16[:], in0=gath[:, :, 0:DIM],
                    in1=gath[:, 0:1, 0:DIM].to_broadcast((128, 16, DIM)),
                    op=mybir.AluOpType.mult)
                dot16 = sp.tile([128, 16], F32, tag="dot16")
                nc.vector.tensor_reduce(dot16[:], prod16[:],
                                        axis=mybir.AxisListType.X,
                                        op=mybir.AluOpType.add)
                # mirror the reference's rounding: d2 = (qn + en) - 2*dot;
                # select on nd2 = 2*dot - (qn + en) == -d2 exactly.
                t16 = sp.tile([128, 16], F32, tag="t16")
                nc.vector.tensor_scalar(out=t16[:], in0=gath[:, :, DIM],
                                        scalar1=qn_s, scalar2=None,
                                        op0=mybir.AluOpType.add)
                s16 = sp.tile([128, 16], F32, tag="s16")
                nc.vector.scalar_tensor_tensor(
                    out=s16[:], in0=dot16[:], scalar=2.0,
                    in1=t16[:],
                    op0=mybir.AluOpType.mult, op1=mybir.AluOpType.subtract)

                # u_k = e_k . f_q for all 16
                prodf = sp.tile([128, 16, DIM], F32, tag="prodf")
                nc.vector.tensor_tensor(
                    out=prodf[:], in0=gath[:, :, 0:DIM],
                    in1=f_s.rearrange("p (o d) -> p o d", o=1).to_broadcast(
                        (128, 16, DIM)),
                    op=mybir.AluOpType.mult)
                u16 = sp.tile([128, 16], F32, tag="u16")
                nc.vector.tensor_reduce(u16[:], prodf[:],
                                        axis=mybir.AxisListType.X,
                                        op=mybir.AluOpType.add)

                # exact top-9, drop rank-1 (self). Winners then selected
                # by MARKING: match_replace flags the first occurrence of
                # each of the 8 w8 values (tie-exact, duplicate-safe, same
                # semantics as position extraction) -> sentinel mask.
                m1 = sp.tile([128, 1], F32, tag="m1")
                nc.vector.tensor_reduce(m1[:], s16[:], axis=mybir.AxisListType.X,
                                        op=mybir.AluOpType.max)
                m1x8 = sp.tile([128, 8], F32, tag="m1x8")
                nc.vector.tensor_copy(m1x8[:], neg_inf8[:])
                nc.vector.tensor_copy(m1x8[:, 0:1], m1[:])
                srep = sp.tile([128, 16], F32, tag="srep")
                nc.vector.match_replace(srep[:], m1x8[:], s16[:], -1e30)
                w8 = sp.tile([128, 8], F32, tag="w8")
                nc.vector.max(w8[:], srep[:])
                srep2 = sp.tile([128, 16], F32, tag="srep2")
                nc.vector.match_replace(srep2[:], w8[:], srep[:], 1e30)
                mask = sp.tile([128, 16], F32, tag="mask")
                nc.vector.tensor_scalar(out=mask[:], in0=srep2[:],
                                        scalar1=1e29, scalar2=None,
                                        op0=mybir.AluOpType.is_ge)

                # dist/weights over all 16, masked; clamp s16 <= 0 first
                # (self's exact s16 can round slightly positive -> sqrt NaN)
                s16c = sp.tile([128, 16], F32, tag="s16c")
                nc.vector.tensor_scalar(out=s16c[:], in0=s16[:], scalar1=0.0,
                                        scalar2=None, op0=mybir.AluOpType.min)
                dist16 = sp.tile([128, 16], F32, tag="dist16")
                nc.scalar.activation(dist16[:], s16c[:],
                                     mybir.ActivationFunctionType.Sqrt,
                                     bias=0.0, scale=-1.0)
                wexp16 = sp.tile([128, 16], F32, tag="wexp16")
                nc.scalar.activation(wexp16[:], dist16[:],
                                     mybir.ActivationFunctionType.Exp,
                                     bias=1.0, scale=-1.0)
                wm16 = sp.tile([128, 16], F32, tag="wm16")
                nc.vector.tensor_tensor(out=wm16[:], in0=wexp16[:], in1=mask[:],
                                        op=mybir.AluOpType.mult)
                scrap16 = sp.tile([128, 16], F32, tag="scrap16")
                nc.vector.scalar_tensor_tensor(
                    out=scrap16[:], in0=u16[:], scalar=qf_s, in1=wm16[:],
                    op0=mybir.AluOpType.subtract, op1=mybir.AluOpType.mult,
                    accum_out=numneg_all[:, m:m + 1])
                nc.vector.tensor_reduce(wsum_all[:, m:m + 1], wm16[:],
                                        axis=mybir.AxisListType.X,
                                        op=mybir.AluOpType.add)

                # debug: (gid+1)*mask so the test can recover the winner set
                gdbg = sp.tile([128, 16], F32, tag="gdbg")
                nc.vector.scalar_tensor_tensor(
                    out=gdbg[:], in0=wgid_f[:], scalar=1.0, in1=mask[:],
                    op0=mybir.AluOpType.add, op1=mybir.AluOpType.mult)
                gdbg_u = sp.tile([128, 16], U32, tag="gdbgu")
                nc.vector.tensor_copy(gdbg_u[:], gdbg[:])

                nc.sync.dma_start(out=dbg_gid_d[m], in_=gdbg_u[:])
                nc.sync.dma_start(out=dbg_s_d[m], in_=w8[:])

            # combine heads: pred = sigmoid(mean_h num_h / den_h)
            nsum2 = sp.tile([128, N_HEADS], F32, tag="nsum2")
            nc.vector.tensor_reduce(
                nsum2[:], numneg_all[:].rearrange("p (h e) -> p h e", e=2),
                axis=mybir.AxisListType.X, op=mybir.AluOpType.add)
            den = sp.tile([128, N_HEADS], F32, tag="den")
            nc.vector.tensor_reduce(
                den[:], wsum_all[:].rearrange("p (h e) -> p h e", e=2),
                axis=mybir.AxisListType.X, op=mybir.AluOpType.add)
            den8 = sp.tile([128, N_HEADS], F32, tag="den8")
            nc.vector.tensor_scalar(out=den8[:], in0=den[:],
                                    scalar1=float(N_SENT), scalar2=None,
                                    op0=mybir.AluOpType.add)
            rden = sp.tile([128, N_HEADS], F32, tag="rden")
            nc.vector.reciprocal(rden[:], den8[:])
            ratio = sp.tile([128, N_HEADS], F32, tag="ratio")
            nc.vector.tensor_tensor(out=ratio[:], in0=nsum2[:], in1=rden[:],
                                    op=mybir.AluOpType.mult)
            ssum = sp.tile([128, 1], F32, tag="ssum")
            nc.vector.tensor_reduce(ssum[:], ratio[:], axis=mybir.AxisListType.X,
                                    op=mybir.AluOpType.add)
            preds_s = sp.tile([128, 1], F32, tag="preds")
            nc.scalar.activation(preds_s[:], ssum[:],
                                 mybir.ActivationFunctionType.Sigmoid,
                                 bias=0.0, scale=-1.0 / N_HEADS)
            nc.sync.dma_start(out=preds_d[:], in_=preds_s[:])

    return nc


def _prep_inputs(embeds, field, edges):
    """Host-side layout prep + per-core sharding."""
    embeds = np.asarray(embeds, dtype=np.float32)
    field = np.asarray(field, dtype=np.float32)
    edges = np.asarray(edges)

    en = np.sum(np.square(embeds), axis=-1, dtype=np.float32)
    rhs_aug = np.empty((N_HEADS, KC, N_PAD), dtype=np.float32)
    rhs_aug[:, :DIM, :N_NODES] = embeds.transpose(0, 2, 1)
    rhs_aug[:, DIM, :N_NODES] = en
    rhs_aug[:, :DIM, N_NODES:] = 0.0
    rhs_aug[:, DIM, N_NODES:] = 1e9    # pad columns get s = -1e9

    embn = np.zeros((N_HEADS * N_NODES, EW), dtype=np.float32)
    embn[:, :DIM] = embeds.reshape(-1, DIM)
    embn[:, DIM] = en.reshape(-1)

    in_maps = []
    for c in range(N_CORES):
        sl = slice(c * EDGES_PER_CORE, (c + 1) * EDGES_PER_CORE)
        qpack = np.zeros((M_TILES, KC, 128), dtype=np.float32)
        aux = np.zeros((M_TILES, 128, DIM + 2), dtype=np.float32)
        for m in range(M_TILES):
            h, e = m // 2, m % 2
            nodes = edges[e, sl]
            q = embeds[h, nodes]                      # (128, 32)
            f = field[h, nodes]                       # (128, 32)
            qpack[m, :DIM] = (2.0 * q).T
            qpack[m, DIM] = -1.0
            aux[m, :, 0] = np.einsum('bd,bd->b', q, q)
            aux[m, :, 1] = np.einsum('bd,bd->b', q, f)
            aux[m, :, 2:] = f
        in_maps.append({
            "rhs_aug": rhs_aug, "embn": embn,
            "qpack": qpack, "aux": aux,
        })
    return in_maps


def kernel(embeds, field, edges):
    from concourse.bass_utils import run_bass_kernel_spmd

    nc = _build_program()
    nc.finalize()
    in_maps = _prep_inputs(embeds, field, edges)
    core_ids = list(range(N_CORES))
    trace = bool(os.environ.get("KNN_TRACE"))
    tmpdir = os.environ.get("KNN_TRACE_DIR") or None
    out = run_bass_kernel_spmd(nc, in_maps, core_ids, trace=trace,
                               tmpdir=tmpdir)
    LAST["results"] = out
    preds = np.concatenate(
        [out.results[c]["preds"][:, 0] for c in range(N_CORES)])
    return preds.astype(np.float32)

